# revision 3
# baseline (speedup 1.0000x reference)
"""Trainium2 Bass kernel for nn_MetaComprehensiveRegularization.

reference math (fp32):
  loss_common  = -sum(zc * zc)
  loss_special = -sum_v sum_i dot(zc_i, zs_vi) / (||zc_i|| * ||zs_vi||)
  output = stack([loss_common, loss_special])  # shape [2]

Strategy: data-parallel shard along N across 8 cores. Each core computes
row-wise sums of squares (ACT Square + accum_out) and row-wise dot products
(DVE tensor_tensor_reduce), combines them into per-partition partial sums
[128, 2], and the host reduces partials across partitions and cores.
"""

import numpy as np

N_CORES = 8
N, D, V = 16384, 512, 4
N_LOC = N // N_CORES      # 2048 rows per core
P = 128                   # SBUF partitions
BLOCKS = N_LOC // P       # 16 row-blocks per core
A = 4                     # row-blocks per DMA chunk (1 MiB per dma_start)
CHUNKS = BLOCKS // A      # 4

_PROGRAM = None


def _build_program():
    import concourse.bacc as bacc
    import concourse.tile as tile
    from concourse import mybir

    f32 = mybir.dt.float32
    nc = bacc.Bacc(
        "TRN2", target_bir_lowering=False, debug=False, num_devices=N_CORES
    )
    zc_t = nc.dram_tensor("zc", [N_LOC, D], f32, kind="ExternalInput")
    zs_t = nc.dram_tensor("zs", [V, N_LOC, D], f32, kind="ExternalInput")
    out_t = nc.dram_tensor("out", [P, 2], f32, kind="ExternalOutput")

    # row n = c*(A*P) + a*P + p  ->  view [c, p, a, d]
    zc_v = zc_t.ap().rearrange("(c a p) d -> c p a d", a=A, p=P)
    zs_v = zs_t.ap().rearrange("v (c a p) d -> v c p a d", a=A, p=P)

    NB = V * BLOCKS  # 64 stat columns for the zs-dependent stats

    with tile.TileContext(nc) as tc:
        with (
            tc.tile_pool(name="zc_pool", bufs=2) as zc_pool,
            tc.tile_pool(name="zs_pool", bufs=8) as zs_pool,
            tc.tile_pool(name="scratch", bufs=2) as scr_pool,
            tc.tile_pool(name="stats", bufs=1) as stats_pool,
        ):
            cn2 = stats_pool.tile([P, BLOCKS], f32)   # ||zc_row||^2 per block
            sn2 = stats_pool.tile([P, NB], f32)       # ||zs_row||^2, col = v*BLOCKS+t
            dot = stats_pool.tile([P, NB], f32)       # <zc,zs_v>,    col = v*BLOCKS+t
            out_sb = stats_pool.tile([P, 2], f32)

            for c in range(CHUNKS):
                zc_tile = zc_pool.tile([P, A, D], f32, tag="zc")
                nc.sync.dma_start(out=zc_tile, in_=zc_v[c])
                for a in range(A):
                    t = c * A + a
                    scr = scr_pool.tile([P, D], f32, tag="scr_c")
                    nc.scalar.activation(
                        out=scr,
                        in_=zc_tile[:, a, :],
                        func=mybir.ActivationFunctionType.Square,
                        accum_out=cn2[:, t : t + 1],
                    )
                for v in range(V):
                    zs_tile = zs_pool.tile([P, A, D], f32, tag="zs")
                    nc.sync.dma_start(out=zs_tile, in_=zs_v[v, c])
                    for a in range(A):
                        t = c * A + a
                        col = v * BLOCKS + t
                        scr2 = scr_pool.tile([P, D], f32, tag="scr_d")
                        nc.vector.scalar_tensor_tensor(
                            out=scr2,
                            in0=zc_tile[:, a, :],
                            scalar=1.0,
                            in1=zs_tile[:, a, :],
                            op0=mybir.AluOpType.mult,
                            op1=mybir.AluOpType.mult,
                            accum_out=dot[:, col : col + 1],
                        )
                        scr3 = scr_pool.tile([P, D], f32, tag="scr_s")
                        nc.scalar.activation(
                            out=scr3,
                            in_=zs_tile[:, a, :],
                            func=mybir.ActivationFunctionType.Square,
                            accum_out=sn2[:, col : col + 1],
                        )

            # cos = dot / sqrt(cn2 * sn2); row norms of randn data are ~sqrt(D)
            # so the reference's eps=1e-12 clamp can never bind.
            prod = stats_pool.tile([P, NB], f32)
            for v in range(V):
                sl = slice(v * BLOCKS, (v + 1) * BLOCKS)
                nc.vector.tensor_mul(out=prod[:, sl], in0=sn2[:, sl], in1=cn2[:, :])
            nc.scalar.activation(
                out=prod, in_=prod, func=mybir.ActivationFunctionType.Sqrt
            )
            rden = stats_pool.tile([P, NB], f32)
            nc.vector.reciprocal(out=rden, in_=prod)
            cosm = stats_pool.tile([P, NB], f32)
            nc.vector.scalar_tensor_tensor(
                out=cosm,
                in0=dot,
                scalar=1.0,
                in1=rden,
                op0=mybir.AluOpType.mult,
                op1=mybir.AluOpType.mult,
                accum_out=out_sb[:, 1:2],
            )
            nc.vector.tensor_reduce(
                out=out_sb[:, 0:1],
                in_=cn2,
                axis=mybir.AxisListType.X,
                op=mybir.AluOpType.add,
            )
            nc.sync.dma_start(out=out_t.ap(), in_=out_sb)

    nc.compile()
    return nc


def _get_program():
    global _PROGRAM
    if _PROGRAM is None:
        _PROGRAM = _build_program()
    return _PROGRAM


def kernel(zc: np.ndarray, zs: np.ndarray) -> np.ndarray:
    from concourse.bass_utils import run_bass_kernel_spmd

    zc = np.ascontiguousarray(np.asarray(zc), dtype=np.float32)
    zs = np.ascontiguousarray(np.asarray(zs), dtype=np.float32)
    assert zc.shape == (N, D) and zs.shape == (V, N, D)

    nc = _get_program()
    in_maps = [
        {
            "zc": np.ascontiguousarray(zc[i * N_LOC : (i + 1) * N_LOC]),
            "zs": np.ascontiguousarray(zs[:, i * N_LOC : (i + 1) * N_LOC]),
        }
        for i in range(N_CORES)
    ]
    res = run_bass_kernel_spmd(nc, in_maps, core_ids=list(range(N_CORES)))
    partials = np.stack([r["out"] for r in res.results])  # [8, 128, 2]
    sums = partials.astype(np.float64).sum(axis=(0, 1))   # [2]
    return np.asarray([-sums[0], -sums[1]], dtype=np.float32)


# revision 4
# speedup vs baseline: 1.0593x; 1.0593x over previous
"""Trainium2 Bass kernel for nn_MetaComprehensiveRegularization.

reference math (fp32):
  loss_common  = -sum(zc * zc)
  loss_special = -sum_v sum_i dot(zc_i, zs_vi) / (||zc_i|| * ||zs_vi||)
  output = stack([loss_common, loss_special])  # shape [2]

Strategy: data-parallel shard along N across 8 cores. Each core computes
row-wise reductions only — ||zc_row||^2 (DVE scalar_tensor_tensor
self-multiply + accum), ||zs_row||^2 (ACT Square + accum_out), and
dot(zc,zs) (DVE scalar_tensor_tensor + accum) — and ships the raw
per-row stats [128, 144] to the host, which combines them into the two
scalars in float64. Work is split so ACT (64 ops) and DVE (80 ops) are
balanced, overlapping the HBM-bound input DMA.
"""

import numpy as np

N_CORES = 8
N, D, V = 16384, 512, 4
N_LOC = N // N_CORES      # 2048 rows per core
P = 128                   # SBUF partitions
BLOCKS = N_LOC // P       # 16 row-blocks per core
A = 4                     # row-blocks per DMA chunk (1 MiB per dma_start)
CHUNKS = BLOCKS // A      # 4
NB = V * BLOCKS           # 64 zs-stat columns
NSTAT = BLOCKS + 2 * NB   # 144 stat columns: [cn2 | sn2 | dot]

_PROGRAM = None


def _build_program():
    import concourse.bacc as bacc
    import concourse.tile as tile
    from concourse import mybir

    f32 = mybir.dt.float32
    nc = bacc.Bacc(
        "TRN2", target_bir_lowering=False, debug=False, num_devices=N_CORES
    )
    zc_t = nc.dram_tensor("zc", [N_LOC, D], f32, kind="ExternalInput")
    zs_t = nc.dram_tensor("zs", [V, N_LOC, D], f32, kind="ExternalInput")
    out_t = nc.dram_tensor("out", [P, NSTAT], f32, kind="ExternalOutput")

    # row n = c*(A*P) + a*P + p  ->  view [c, p, a, d]
    zc_v = zc_t.ap().rearrange("(c a p) d -> c p a d", a=A, p=P)
    zs_v = zs_t.ap().rearrange("v (c a p) d -> v c p a d", a=A, p=P)

    with tile.TileContext(nc) as tc:
        with (
            tc.tile_pool(name="zc_pool", bufs=2) as zc_pool,
            tc.tile_pool(name="zs_pool", bufs=6) as zs_pool,
            tc.tile_pool(name="stats", bufs=1) as stats_pool,
        ):
            stats = stats_pool.tile([P, NSTAT], f32)
            # Dead full-width `out` sinks for the accumulating ops — one per
            # engine so WAW stays within an engine's program order.
            dummy_dve = stats_pool.tile([P, 1], f32)
            dummy_act = stats_pool.tile([P, 1], f32)

            for c in range(CHUNKS):
                zc_tile = zc_pool.tile([P, A, D], f32, tag="zc")
                nc.sync.dma_start(out=zc_tile, in_=zc_v[c])
                for a in range(A):
                    t = c * A + a
                    nc.vector.scalar_tensor_tensor(
                        out=dummy_dve.broadcast_to((P, D)),
                        in0=zc_tile[:, a, :],
                        scalar=1.0,
                        in1=zc_tile[:, a, :],
                        op0=mybir.AluOpType.mult,
                        op1=mybir.AluOpType.mult,
                        accum_out=stats[:, t : t + 1],
                    )
                for v in range(V):
                    zs_tile = zs_pool.tile([P, A, D], f32, tag="zs")
                    nc.sync.dma_start(out=zs_tile, in_=zs_v[v, c])
                    for a in range(A):
                        t = c * A + a
                        col_sn = BLOCKS + v * BLOCKS + t
                        col_dot = BLOCKS + NB + v * BLOCKS + t
                        nc.vector.scalar_tensor_tensor(
                            out=dummy_dve.broadcast_to((P, D)),
                            in0=zc_tile[:, a, :],
                            scalar=1.0,
                            in1=zs_tile[:, a, :],
                            op0=mybir.AluOpType.mult,
                            op1=mybir.AluOpType.mult,
                            accum_out=stats[:, col_dot : col_dot + 1],
                        )
                        nc.scalar.activation(
                            out=dummy_act.broadcast_to((P, D)),
                            in_=zs_tile[:, a, :],
                            func=mybir.ActivationFunctionType.Square,
                            accum_out=stats[:, col_sn : col_sn + 1],
                        )

            nc.sync.dma_start(out=out_t.ap(), in_=stats)

    nc.compile()
    return nc


def _get_program():
    global _PROGRAM
    if _PROGRAM is None:
        _PROGRAM = _build_program()
    return _PROGRAM


def _combine(stats: np.ndarray) -> tuple[float, float]:
    """stats: [n_cores, P, NSTAT] fp32 -> (sum zc^2, sum cosines) in fp64."""
    s = stats.astype(np.float64)
    cn2 = s[:, :, :BLOCKS]                          # [cores, P, 16]
    sn2 = s[:, :, BLOCKS : BLOCKS + NB]             # [cores, P, 64]
    dot = s[:, :, BLOCKS + NB :]                    # [cores, P, 64]
    common = cn2.sum()
    eps = 1e-12
    cn = np.maximum(np.sqrt(cn2), eps)              # [cores, P, 16]
    sn = np.maximum(np.sqrt(sn2), eps)              # [cores, P, 64]
    v_cn = np.tile(cn, (1, 1, V))                   # align with v*16+t layout
    special = (dot / (v_cn * sn)).sum()
    return common, special


def kernel(zc: np.ndarray, zs: np.ndarray) -> np.ndarray:
    from concourse.bass_utils import run_bass_kernel_spmd

    zc = np.ascontiguousarray(np.asarray(zc), dtype=np.float32)
    zs = np.ascontiguousarray(np.asarray(zs), dtype=np.float32)
    assert zc.shape == (N, D) and zs.shape == (V, N, D)

    nc = _get_program()
    in_maps = [
        {
            "zc": np.ascontiguousarray(zc[i * N_LOC : (i + 1) * N_LOC]),
            "zs": np.ascontiguousarray(zs[:, i * N_LOC : (i + 1) * N_LOC]),
        }
        for i in range(N_CORES)
    ]
    res = run_bass_kernel_spmd(nc, in_maps, core_ids=list(range(N_CORES)))
    stats = np.stack([r["out"] for r in res.results])  # [8, 128, 144]
    common, special = _combine(stats)
    return np.asarray([-common, -special], dtype=np.float32)
